# revision 10
# baseline (speedup 1.0000x reference)
"""MultiHeadLinearAttention Trainium2 kernel (8-core SPMD, fp32r matmuls).

Sharding: 16384 tokens split across 8 cores (core c: batch c//2, sequence half
c%2). All projections/attention/out-proj are local; the only cross-core
dependency is the per-batch KV summary (kv [H,DK,DK] + ksum [D]) reduced via a
266KB pair-wise AllReduce, overlapped with the q-projection.

Layouts (all chosen so no transposes are ever needed on device):
  - host pre-transposes x (feature-major xT [D,T]) and weights (wT [din,dout])
  - q GLU computed feature-major (bias per-partition via ACT)
  - k/v GLU computed token-major (bias via K=1 ones-outer matmul into PSUM)
  - kv/ksum contraction over tokens (token-major phi_k/v)
  - z via block-diag ksum lhsT; 1/(z+eps) broadcast across partitions via
    K=1 outer-product; applied at num-eviction (fused DVE multiply)
  - out-proj consumes feature-major attn directly
"""
from contextlib import ExitStack

import numpy as np
import concourse.mybir as mybir
import concourse.tile as tile
from concourse import bacc
from concourse.tile import add_dep_helper
from concourse.bass_utils import run_bass_kernel_spmd

F32 = mybir.dt.float32
F32R = mybir.dt.float32r
ACTF = mybir.ActivationFunctionType
ALU = mybir.AluOpType

B, S, D, H = 4, 4096, 1024, 16
DK = D // H          # 64
EPS = 1e-6
NCORES = 8
T = B * S // NCORES  # 2048 tokens per core
P = 128
NM = T // P          # 16 token tiles
NCD = D // P         # 8 d-chunks
CH = 256             # stage-2 token chunk
NCH = T // CH        # 8 chunks
GROUPS = [[0, 1], [2, 3], [4, 5], [6, 7]]


def build(single_core=False):
    nc = bacc.Bacc("TRN2", target_bir_lowering=False, debug=False,
                   num_devices=1 if single_core else NCORES)
    dt_in = {}

    def inp(name, shape):
        dt_in[name] = nc.dram_tensor(name, shape, F32, kind="ExternalInput").ap()
        return dt_in[name]

    xqT = inp("xqT", [D, T])
    xkT = inp("xkT", [D, T])
    xvT = inp("xvT", [D, T])
    wq1T = inp("wq1T", [D, D]); wq2T = inp("wq2T", [D, D])
    wk1T = inp("wk1T", [D, D]); wk2T = inp("wk2T", [D, D])
    wv1T = inp("wv1T", [D, D]); wv2T = inp("wv2T", [D, D])
    woT = inp("woT", [D, D])
    bq1 = inp("bq1", [D]); bq2 = inp("bq2", [D])
    bk1 = inp("bk1", [D]); bk2 = inp("bk2", [D])
    bv1 = inp("bv1", [D]); bv2 = inp("bv2", [D])
    bo = inp("bo", [D])
    ones_row = inp("ones_row", [1, P])
    zeros16 = inp("zeros16", [P, H])
    maskc = inp("maskc", [T])
    out = nc.dram_tensor("out", [T, D], F32, kind="ExternalOutput").ap()

    with tile.TileContext(nc) as tc:
        _emit(nc, tc, dt_in, out, single_core)
    nc.compile()
    return nc


def _emit(nc, tc, dt, out, single_core):
    def mm(psum, lhsT, rhs, start, stop, skip=False):
        nc.tensor.matmul(psum, lhsT, rhs, start=start, stop=stop,
                         skip_group_check=skip)

    with ExitStack() as st0:
        const = st0.enter_context(tc.tile_pool(name="const", bufs=1))
        dram = st0.enter_context(tc.tile_pool(name="dram", bufs=1, space="DRAM"))
        kvres = st0.enter_context(tc.tile_pool(name="kvres", bufs=1))
        ones_sb = const.tile([1, P], F32R, tag="ones", name="ones")
        nc.sync.dma_start(ones_sb[:], dt["ones_row"][:].bitcast(F32R))
        ones_col = const.tile([P, 1], F32R, tag="ones_col", name="ones_col")
        nc.sync.dma_start(ones_col[:], dt["ones_row"][:].rearrange("a t -> t a").bitcast(F32R))
        mask_sb = const.tile([P, NM], F32, tag="mask", name="mask")
        nc.sync.dma_start(mask_sb[:], dt["maskc"][:].rearrange("(m p) -> p m", p=P))
        brow = {}
        for nm in ("bk1", "bk2", "bv1", "bv2", "bo"):
            brow[nm] = const.tile([1, D], F32R, tag=f"row_{nm}", name=f"row_{nm}")
            nc.sync.dma_start(brow[nm][:], dt[nm][:].rearrange("(a d) -> a d", a=1).bitcast(F32R))
        bcol = {}
        for nm in ("bq1", "bq2"):
            bcol[nm] = const.tile([P, NCD], F32, tag=f"col_{nm}", name=f"col_{nm}")
            nc.sync.dma_start(bcol[nm][:], dt[nm][:].rearrange("(c p) -> p c", p=P))

        st1 = st0.enter_context(ExitStack())
        phik_pool = st1.enter_context(tc.tile_pool(name="phik", bufs=1))
        phi_k = [phik_pool.tile([P, D], F32R, tag=f"phik_{m}", name=f"phik_{m}") for m in range(NM)]

        # ---------------- stage 1a: k projection -> phi_k ----------------
        with ExitStack() as st1a:
            wkp = st1a.enter_context(tc.tile_pool(name="wk", bufs=1))
            xkp = st1a.enter_context(tc.tile_pool(name="xk", bufs=3))
            t1a = st1a.enter_context(tc.tile_pool(name="t1a", bufs=2))
            pk1p = st1a.enter_context(tc.tile_pool(name="pk1", bufs=2, space="PSUM"))
            pk2p = st1a.enter_context(tc.tile_pool(name="pk2", bufs=2, space="PSUM"))
            pksp = st1a.enter_context(tc.tile_pool(name="pks", bufs=1, space="PSUM"))
            wk_sb = {}
            for w, src in (("w1", "wk1T"), ("w2", "wk2T")):
                for c in range(NCD):
                    wk_sb[w, c] = wkp.tile([P, D], F32R, tag=f"wk_{w}_{c}", name=f"wk_{w}_{c}")
                    nc.sync.dma_start(wk_sb[w, c][:],
                                      dt[src][c * P:(c + 1) * P, :].bitcast(F32R))
            psum_ks = [pksp.tile([1, 512], F32, tag=f"ks{i}", name=f"ks{i}") for i in range(2)]

            for m in range(NM):
                xk_m = xkp.tile([P, D], F32R, tag="xk", name="xk")
                for c in range(NCD):
                    nc.sync.dma_start(
                        xk_m[:, c * P:(c + 1) * P],
                        dt["xkT"][c * P:(c + 1) * P, m * P:(m + 1) * P].bitcast(F32R))
                for n in range(2):
                    ns = slice(n * 512, (n + 1) * 512)
                    p1 = pk1p.tile([P, 512], F32, tag="pk1", name="pk1")
                    p2 = pk2p.tile([P, 512], F32, tag="pk2", name="pk2")
                    for c in range(NCD):
                        mm(p1[:], xk_m[:, c * P:(c + 1) * P], wk_sb["w1", c][:, ns],
                           start=(c == 0), stop=False)
                    mm(p1[:], ones_sb[:], brow["bk1"][0:1, ns], start=False, stop=True)
                    for c in range(NCD):
                        mm(p2[:], xk_m[:, c * P:(c + 1) * P], wk_sb["w2", c][:, ns],
                           start=(c == 0), stop=False)
                    mm(p2[:], ones_sb[:], brow["bk2"][0:1, ns], start=False, stop=True)
                    a1 = t1a.tile([P, 512], F32, tag="a1", name="a1")
                    nc.scalar.activation(a1[:], p1[:], ACTF.Sigmoid)
                    g1 = t1a.tile([P, 512], F32, tag="g1", name="g1")
                    nc.vector.tensor_tensor(g1[:], a1[:], p1[:], ALU.mult)
                    kg = t1a.tile([P, 512], F32, tag="kg", name="kg")
                    nc.vector.tensor_tensor(kg[:], g1[:], p2[:], ALU.mult)
                    tmin = t1a.tile([P, 512], F32, tag="tmin", name="tmin")
                    nc.vector.tensor_scalar_min(tmin[:], kg[:], 0.0)
                    texp = t1a.tile([P, 512], F32, tag="texp", name="texp")
                    nc.scalar.activation(texp[:], tmin[:], ACTF.Exp)
                    trel = t1a.tile([P, 512], F32, tag="trel", name="trel")
                    nc.scalar.activation(trel[:], kg[:], ACTF.Relu,
                                         scale=mask_sb[:, m:m + 1])
                    # phi_k = exp(min(kg,0))*mask + relu(kg*mask)
                    nc.vector.scalar_tensor_tensor(
                        phi_k[m][:, ns], texp[:], mask_sb[:, m:m + 1], trel[:],
                        ALU.mult, ALU.add)
                # ksum row: [1, 1024] += ones_col.T @ phi_k[m]
                for i in range(2):
                    mm(psum_ks[i][:], ones_col[:],
                       phi_k[m][:, i * 512:(i + 1) * 512],
                       start=(m == 0), stop=(m == NM - 1))

            cc_ks_sb = kvres.tile([1, D], F32, tag="cc_ks_sb", name="cc_ks_sb")
            for i in range(2):
                nc.scalar.activation(cc_ks_sb[0:1, i * 512:(i + 1) * 512],
                                     psum_ks[i][:], ACTF.Copy)

        # ---------------- stage 1b: v projection + kv accumulation ----------------
        with ExitStack() as st1b:
            wvp = st1b.enter_context(tc.tile_pool(name="wv", bufs=1))
            xvp = st1b.enter_context(tc.tile_pool(name="xv", bufs=3))
            t1b = st1b.enter_context(tc.tile_pool(name="t1b", bufs=2))
            pv1p = st1b.enter_context(tc.tile_pool(name="pv1", bufs=2, space="PSUM"))
            pv2p = st1b.enter_context(tc.tile_pool(name="pv2", bufs=2, space="PSUM"))
            pkvp = st1b.enter_context(tc.tile_pool(name="pkv", bufs=2, space="PSUM"))
            wv_sb = {}
            for w, src in (("w1", "wv1T"), ("w2", "wv2T")):
                for c in range(NCD):
                    wv_sb[w, c] = wvp.tile([P, D], F32R, tag=f"wv_{w}_{c}", name=f"wv_{w}_{c}")
                    nc.sync.dma_start(wv_sb[w, c][:],
                                      dt[src][c * P:(c + 1) * P, :].bitcast(F32R))
            kv_acc = [kvres.tile([64, 512], F32, tag=f"kv_acc{i}", name=f"kv_acc{i}")
                      for i in range(2)]

            for m in range(NM):
                xv_m = xvp.tile([P, D], F32R, tag="xv", name="xv")
                for c in range(NCD):
                    nc.sync.dma_start(
                        xv_m[:, c * P:(c + 1) * P],
                        dt["xvT"][c * P:(c + 1) * P, m * P:(m + 1) * P].bitcast(F32R))
                vg = t1b.tile([P, D], F32R, tag="vg", name="vg")
                for n in range(2):
                    ns = slice(n * 512, (n + 1) * 512)
                    p1 = pv1p.tile([P, 512], F32, tag="pv1", name="pv1")
                    p2 = pv2p.tile([P, 512], F32, tag="pv2", name="pv2")
                    for c in range(NCD):
                        mm(p1[:], xv_m[:, c * P:(c + 1) * P], wv_sb["w1", c][:, ns],
                           start=(c == 0), stop=False)
                    mm(p1[:], ones_sb[:], brow["bv1"][0:1, ns], start=False, stop=True)
                    for c in range(NCD):
                        mm(p2[:], xv_m[:, c * P:(c + 1) * P], wv_sb["w2", c][:, ns],
                           start=(c == 0), stop=False)
                    mm(p2[:], ones_sb[:], brow["bv2"][0:1, ns], start=False, stop=True)
                    a1 = t1b.tile([P, 512], F32, tag="va1", name="va1")
                    nc.scalar.activation(a1[:], p1[:], ACTF.Sigmoid)
                    g1 = t1b.tile([P, 512], F32, tag="vg1", name="vg1")
                    nc.vector.tensor_tensor(g1[:], a1[:], p1[:], ALU.mult)
                    nc.vector.tensor_tensor(vg[:, ns], g1[:], p2[:], ALU.mult)
                # kv[h] += phi_k[m][:, h]^T @ vg[:, h]; one complete PSUM group
                # per (m, bank-half), accumulated into SBUF (start=True clears
                # has_written for the whole bank, so groups can't interleave).
                for i in range(2):
                    pkv = pkvp.tile([64, 512], F32, tag="pkv", name="pkv")
                    prev = None
                    for j in range(8):
                        h = i * 8 + j
                        hs = slice(h * DK, (h + 1) * DK)
                        inst = nc.tensor.matmul(
                            pkv[0:64, j * DK:(j + 1) * DK],
                            phi_k[m][:, hs], vg[:, hs],
                            start=(j == 0), stop=(j == 7))
                        prev = inst  # PE executes matmuls in program order
                    if m == 0:
                        nc.vector.tensor_copy(kv_acc[i][:], pkv[:])
                    else:
                        nc.vector.tensor_tensor(kv_acc[i][:], kv_acc[i][:], pkv[:],
                                                ALU.add)

        st1.close()  # frees phi_k SBUF before stage 2
        # ---------------- collective: pair AllReduce of kv + ksum ----------------
        cc_in = dram.tile([130, 512], F32)
        cc_out = dram.tile([130, 512], F32)
        nc.sync.dma_start(cc_in[0:64, :], kv_acc[0][:])
        nc.sync.dma_start(cc_in[64:128, :], kv_acc[1][:])
        nc.sync.dma_start(cc_in[128:130, :], cc_ks_sb[:])
        if single_core:
            nc.sync.dma_start(cc_out[:], cc_in[:])
        else:
            nc.gpsimd.collective_compute(
                "AllReduce", ALU.add, replica_groups=GROUPS,
                ins=[cc_in.opt()], outs=[cc_out.opt()])

        # reduced kv -> pair-packed sbuf tile; ksum -> block-diag lhsT tiles
        kv_pairs = kvres.tile([P, 512], F32R, tag="kv_pairs", name="kv_pairs")
        for h in range(H):
            r0 = 0 if h < 8 else 64
            nc.sync.dma_start(
                kv_pairs[(h % 2) * 64:(h % 2) * 64 + 64,
                         (h // 2) * DK:(h // 2 + 1) * DK],
                cc_out[r0:r0 + 64, (h % 8) * DK:(h % 8 + 1) * DK].bitcast(F32R))
        ksum_bd = []
        for c in range(NCD):
            bd = kvres.tile([P, H], F32R, tag=f"bd{c}", name=f"bd{c}")
            nc.sync.dma_start(bd[:], dt["zeros16"][:].bitcast(F32R))
            # ksum[d] lives at cc_out[128 + d // 512, d % 512]
            d0 = c * P
            nc.sync.dma_start(
                bd[0:64, 2 * c:2 * c + 1],
                cc_out[128 + d0 // 512:129 + d0 // 512,
                       d0 % 512:d0 % 512 + 64].bitcast(F32R))
            d1 = c * P + 64
            nc.sync.dma_start(
                bd[64:128, 2 * c + 1:2 * c + 2],
                cc_out[128 + d1 // 512:129 + d1 // 512,
                       d1 % 512:d1 % 512 + 64].bitcast(F32R))
            ksum_bd.append(bd)

        # ---------------- stage 2: q -> phi_q -> z -> attn -> out ----------------
        with ExitStack() as st2:
            wqp = st2.enter_context(tc.tile_pool(name="wq", bufs=1))
            wop = st2.enter_context(tc.tile_pool(name="wo", bufs=1))
            xqp = st2.enter_context(tc.tile_pool(name="xq", bufs=2))
            phiqp = st2.enter_context(tc.tile_pool(name="phiq", bufs=1))
            attnp = st2.enter_context(tc.tile_pool(name="attn", bufs=1))
            t2 = st2.enter_context(tc.tile_pool(name="t2", bufs=2))
            rrp = st2.enter_context(tc.tile_pool(name="rr", bufs=2))
            osbp = st2.enter_context(tc.tile_pool(name="osb", bufs=2))
            pq1p = st2.enter_context(tc.tile_pool(name="pq1", bufs=2, space="PSUM"))
            pq2p = st2.enter_context(tc.tile_pool(name="pq2", bufs=2, space="PSUM"))
            pzp = st2.enter_context(tc.tile_pool(name="pz", bufs=1, space="PSUM"))
            prp = st2.enter_context(tc.tile_pool(name="pr", bufs=1, space="PSUM"))
            pnp = st2.enter_context(tc.tile_pool(name="pn", bufs=1, space="PSUM"))
            pop = st2.enter_context(tc.tile_pool(name="po", bufs=1, space="PSUM"))
            wq_sb = {}
            for w, src in (("w1", "wq1T"), ("w2", "wq2T")):
                for c in range(NCD):
                    wq_sb[w, c] = wqp.tile([P, D], F32R, tag=f"wq_{w}_{c}", name=f"wq_{w}_{c}")
                    nc.sync.dma_start(wq_sb[w, c][:],
                                      dt[src][c * P:(c + 1) * P, :].bitcast(F32R))
            wo_sb = {}
            for c in range(NCD):
                wo_sb[c] = wop.tile([P, D], F32R, tag=f"wo_{c}", name=f"wo_{c}")
                nc.sync.dma_start(wo_sb[c][:],
                                  dt["woT"][c * P:(c + 1) * P, :].bitcast(F32R))

            for ch in range(NCH):
                ts = slice(ch * CH, (ch + 1) * CH)
                xq_ch = xqp.tile([P, NCD * CH], F32R, tag="xq", name="xq")
                for c in range(NCD):
                    nc.sync.dma_start(
                        xq_ch[:, c * CH:(c + 1) * CH],
                        dt["xqT"][c * P:(c + 1) * P, ts].bitcast(F32R))
                phi_q = [phiqp.tile([P, CH], F32R, tag=f"phiq{mc}", name=f"phiq{mc}") for mc in range(NCD)]
                for mc in range(NCD):
                    ms = slice(mc * P, (mc + 1) * P)
                    p1 = pq1p.tile([P, CH], F32, tag="pq1", name="pq1")
                    p2 = pq2p.tile([P, CH], F32, tag="pq2", name="pq2")
                    for c in range(NCD):
                        mm(p1[:], wq_sb["w1", c][:, ms], xq_ch[:, c * CH:(c + 1) * CH],
                           start=(c == 0), stop=(c == NCD - 1))
                    for c in range(NCD):
                        mm(p2[:], wq_sb["w2", c][:, ms], xq_ch[:, c * CH:(c + 1) * CH],
                           start=(c == 0), stop=(c == NCD - 1))
                    a1 = t2.tile([P, CH], F32, tag="qa1", name="qa1")
                    nc.scalar.activation(a1[:], p1[:], ACTF.Sigmoid,
                                         bias=bcol["bq1"][:, mc:mc + 1])
                    s1 = t2.tile([P, CH], F32, tag="qs1", name="qs1")
                    nc.vector.scalar_tensor_tensor(s1[:], p1[:],
                                                   bcol["bq1"][:, mc:mc + 1], a1[:],
                                                   ALU.add, ALU.mult)
                    qg = t2.tile([P, CH], F32, tag="qg", name="qg")
                    nc.vector.scalar_tensor_tensor(qg[:], p2[:],
                                                   bcol["bq2"][:, mc:mc + 1], s1[:],
                                                   ALU.add, ALU.mult)
                    tmin = t2.tile([P, CH], F32, tag="qtmin", name="qtmin")
                    nc.vector.tensor_scalar_min(tmin[:], qg[:], 0.0)
                    texp = t2.tile([P, CH], F32, tag="qtexp", name="qtexp")
                    nc.scalar.activation(texp[:], tmin[:], ACTF.Exp)
                    trel = t2.tile([P, CH], F32, tag="qtrel", name="qtrel")
                    nc.scalar.activation(trel[:], qg[:], ACTF.Relu)
                    nc.vector.tensor_tensor(phi_q[mc][:], texp[:], trel[:], ALU.add)

                # z[h, s] then r = 1/(z+eps), gathered to one row
                pz = pzp.tile([H, CH], F32, tag="pz", name="pz")
                for c in range(NCD):
                    mm(pz[:], ksum_bd[c][:], phi_q[c][:],
                       start=(c == 0), stop=(c == NCD - 1))
                zeps = t2.tile([H, CH], F32, tag="zeps", name="zeps")
                nc.vector.tensor_scalar_add(zeps[:], pz[:], EPS)
                r_sb = t2.tile([H, CH], F32, tag="r_sb", name="r_sb")
                nc.vector.reciprocal(r_sb[:], zeps[:])
                attn = [attnp.tile([P, CH], F32R, tag=f"attn{c}", name=f"attn{c}") for c in range(NCD)]
                for pair in range(NCD):
                    r2 = t2.tile([1, 2 * CH], F32R, tag="r2", name="r2")
                    nc.sync.dma_start(r2[:], r_sb[2 * pair:2 * pair + 2, :].bitcast(F32R))
                    for hb, h in ((0, 2 * pair), (64, 2 * pair + 1)):
                        pr = prp.tile([64, CH], F32, tag="pr", name="pr")
                        mm(pr[:], ones_sb[0:1, 0:64],
                           r2[0:1, (h % 2) * CH:((h % 2) + 1) * CH],
                           start=True, stop=True)
                        r_rep = rrp.tile([64, CH], F32, tag="r_rep", name="r_rep")
                        nc.scalar.activation(r_rep[:], pr[:], ACTF.Copy)
                        pn = pnp.tile([64, CH], F32, tag="pn", name="pn")
                        mm(pn[:], kv_pairs[hb:hb + 64, pair * DK:(pair + 1) * DK],
                           phi_q[pair][hb:hb + 64, :], start=True, stop=True)
                        nc.vector.tensor_tensor(attn[pair][hb:hb + 64, :],
                                                pn[:], r_rep[:], ALU.mult)

                # out projection for this chunk
                for mt in range(CH // P):
                    o_sb = osbp.tile([P, D], F32, tag="o_sb", name="o_sb")
                    for n in range(2):
                        ns = slice(n * 512, (n + 1) * 512)
                        po = pop.tile([P, 512], F32, tag="po", name="po")
                        for c in range(NCD):
                            mm(po[:], attn[c][:, mt * P:(mt + 1) * P],
                               wo_sb[c][:, ns], start=(c == 0), stop=False)
                        mm(po[:], ones_sb[:], brow["bo"][0:1, ns],
                           start=False, stop=True)
                        nc.scalar.activation(o_sb[:, ns], po[:], ACTF.Copy)
                    row0 = ch * CH + mt * P
                    nc.sync.dma_start(out[row0:row0 + P, :], o_sb[:])


_CACHE = {}


def _get_nc(single_core=False):
    key = bool(single_core)
    if key not in _CACHE:
        _CACHE[key] = build(single_core)
    return _CACHE[key]


def make_in_maps(inputs):
    f = np.float32
    q = np.asarray(inputs["query"], f).reshape(B * S, D)
    k = np.asarray(inputs["key"], f).reshape(B * S, D)
    v = np.asarray(inputs["value"], f).reshape(B * S, D)
    mask = np.asarray(inputs["mask"], f).reshape(B * S)
    shared = {
        "wq1T": np.ascontiguousarray(np.asarray(inputs["q_w1"], f).T),
        "wq2T": np.ascontiguousarray(np.asarray(inputs["q_w2"], f).T),
        "wk1T": np.ascontiguousarray(np.asarray(inputs["k_w1"], f).T),
        "wk2T": np.ascontiguousarray(np.asarray(inputs["k_w2"], f).T),
        "wv1T": np.ascontiguousarray(np.asarray(inputs["v_w1"], f).T),
        "wv2T": np.ascontiguousarray(np.asarray(inputs["v_w2"], f).T),
        "woT": np.ascontiguousarray(np.asarray(inputs["out_w"], f).T),
        "bq1": np.asarray(inputs["q_b1"], f), "bq2": np.asarray(inputs["q_b2"], f),
        "bk1": np.asarray(inputs["k_b1"], f), "bk2": np.asarray(inputs["k_b2"], f),
        "bv1": np.asarray(inputs["v_b1"], f), "bv2": np.asarray(inputs["v_b2"], f),
        "bo": np.asarray(inputs["out_b"], f),
        "ones_row": np.ones((1, P), f),
        "zeros16": np.zeros((P, H), f),
    }
    in_maps = []
    for c in range(NCORES):
        sl = slice(c * T, (c + 1) * T)
        m = dict(shared)
        m["xqT"] = np.ascontiguousarray(q[sl].T)
        m["xkT"] = np.ascontiguousarray(k[sl].T)
        m["xvT"] = np.ascontiguousarray(v[sl].T)
        m["maskc"] = np.ascontiguousarray(mask[sl])
        in_maps.append(m)
    return in_maps


def kernel(**inputs):
    nc = _get_nc(False)
    in_maps = make_in_maps(inputs)
    res = run_bass_kernel_spmd(nc, in_maps, list(range(NCORES))).results
    outc = np.concatenate([res[c]["out"] for c in range(NCORES)], axis=0)
    return outc.reshape(B, S, D)
